# revision 74
# baseline (speedup 1.0000x reference)
"""Attention2d Trainium2 kernel.

Reference computation (per sample b):
  K = Wk @ x;  Q = Wq @ x + bq;  V = Wv @ x + bv     (x: [128, 1024])
  per head h (32 channels):  att[k,q] = scale * K_h[:,k].Q_h[:,q] + rel_h[k,q]
  P = softmax_k(att);  out_h = V_h @ P;  y = Wu @ out + bu

Kernel strategy (8 NeuronCores, data-parallel over batch, 2 samples/core):
  - exp(att + rel) = exp(att) * exp(rel).  exp(rel) is gathered host-side
    (exp commutes with the pos_enc gather) and uploaded fp16; the multiply
    runs on the vector/gpsimd engines in SBUF.  This removes the rel-add
    identity matmuls from the PE entirely (1/3 of its matmul work).
  - exp(att) tiles are computed uniformly scaled by 4: ScalarE tiles use
    ACT Exp with bias=ln(4) -> 4*exp(att); a tunable subset of tiles runs
    on the vector engine as (att+2)^2 = 4*(1+att/2)^2 ~= 4*exp(att).  The
    scale cancels exactly in P = E/D, and the DVE path offloads the
    otherwise-bottleneck ScalarE.
  - AV matmul packs a head pair into one [66, N] PSUM accumulator via
    zero-padded stationaries [V_h0|0|ones|0] and [0|V_h1|0|ones]: channel
    rows 0..63 and both softmax denominators D at rows 64..65, so one copy
    evacuates the pair and the reciprocal reads D straight from PSUM.
  - division by D applied pre-Wu via a selector-matmul partition-broadcast
    of 1/D (softmax denominator), bv/bu folded into one final bias.
"""

import math
import os
import sys
import types

sys.path.insert(0, "/opt/trn_rl_repo")

import numpy as np

import concourse.bass as bass
import concourse.tile as tile
from concourse import bacc, mybir
from concourse import bass_utils
from concourse.bass import ds, ts

F32 = mybir.dt.float32
F16 = mybir.dt.float16
AF = mybir.ActivationFunctionType
ALU = mybir.AluOpType

B, E, H, NY, NX = 16, 128, 4, 32, 32
N = NY * NX          # 1024
HC = E // H          # 32
NCORES = 8
BPC = B // NCORES    # 2 samples per core
NT = N // 128        # 8 k-tiles
SCALE = HC ** -0.5
LN4 = math.log(4.0)

LAST_RESULT = None   # BassKernelResults of the most recent run (for test.py)

_CACHE = {}


def _patch_ldw_opt():
    """Enable walrus LDWEIGHTS elision (redundant identity reloads)."""
    if _CACHE.get("ldw_patched"):
        return
    import concourse.bass_utils as _bu
    orig = _bu.run_command

    def patched(argv, **kw):
        argv = [a.replace("--enable-ldw-opt=false", "--enable-ldw-opt=true")
                if isinstance(a, str) else a for a in argv]
        return orig(argv, **kw)

    _bu.run_command = patched
    _CACHE["ldw_patched"] = True


def _ensure_ntff_hook():
    """Register the axon NTFF profile hook that trn_boot couldn't install
    (the image lacks antenv.axon_hooks). Only needed when tracing."""
    if "antenv.axon_hooks" in sys.modules:
        return
    mod = types.ModuleType("antenv.axon_hooks")
    holder = [None]
    mod.set_axon_ntff_profile_hook = lambda h: holder.__setitem__(0, h)
    mod.get_axon_ntff_profile_hook = lambda: holder[0]
    sys.modules["antenv.axon_hooks"] = mod
    try:
        from trn_agent_boot.trn_boot import _ntff_profile_via_ctypes
        mod.set_axon_ntff_profile_hook(
            _ntff_profile_via_ctypes("/opt/axon/libaxon_pjrt.so")
        )
    except Exception:
        pass


def _rel_indices(ny, nx):
    y = np.arange(ny)
    x = np.arange(nx)
    y1, x1, y2, x2 = np.meshgrid(y, x, y, x, indexing="ij")
    idx = (y1 - y2 + ny - 1) * (2 * nx - 1) + (x1 - x2 + nx - 1)
    return idx.reshape(ny * nx, ny * nx)


def _spread(n, total=64):
    """Bresenham-spread set of n tile indices among `total` units."""
    out = set()
    if n <= 0:
        return out
    for i in range(total):
        if (i * n) // total != ((i + 1) * n) // total:
            out.add(i)
    return out


def _build():
    """Build + bacc-compile the per-core program (cached)."""
    if os.environ.get("KLDW", "0") == "1":
        _patch_ldw_opt()
    nsq = int(os.environ.get("KSQ", "0"))     # tiles on the DVE (x+2)^2 path
    ngp = int(os.environ.get("KMG", "8"))     # Sc-path rel-muls on GpSimd
    lka = int(os.environ.get("KLA", "5"))     # AV lookahead (units)
    nwu = int(os.environ.get("KWU", "36"))    # PE warm-up burst matmuls
    ndm = int(os.environ.get("KDM", "3"))     # warm-keeper matmuls per unit
    bfz = bool(int(os.environ.get("KBFZ", "1")))  # final bias known zero
    kdw = int(os.environ.get("KDW", "384"))   # warm-keeper matmul width
    key = ("nc", nsq, ngp, lka, nwu, ndm, bfz, kdw)
    if key in _CACHE:
        return _CACHE[key]
    _is_h2x = lambda i: ((i % 32) // 16 == 1) and (i % 2 == 0)
    sqset = {i for i in _spread(nsq) if not _is_h2x(i)}
    # GpSimd rel-muls: skip the first 12 tiles (6 units) so the pipe fills
    # without waiting on the slow engine; head-2 tiles are PE rel-add path
    _is_h2 = lambda i: ((i % 32) // 16 == 1) and (i % 2 == 0)
    scpath = [i for i in range(12, 64) if i not in sqset and not _is_h2(i)]
    gpmul = {scpath[i] for i in _spread(min(ngp, len(scpath)), len(scpath))}

    nc = bacc.Bacc("TRN2", target_bir_lowering=False, debug=False,
                   num_devices=NCORES)

    d_x2 = nc.dram_tensor("x2", [BPC, E, N], F16, kind="ExternalInput")
    d_wall = nc.dram_tensor("wall", [E, 4, E], F16, kind="ExternalInput")
    d_bq = nc.dram_tensor("bqv", [E, 1], F32, kind="ExternalInput")
    d_bf = nc.dram_tensor("bfv", [E, 1], F32, kind="ExternalInput")
    # compact Toeplitz rel table: T3[h, p, 1984-252t+63*y2+x2] =
    # exp(rel)[h, 128t+p, 32*y2+x2]  (one shared table for all key-tiles t)
    d_rel = nc.dram_tensor("relb", [H, 128, 4032], F16, kind="ExternalInput")
    d_sel = nc.dram_tensor("sel2", [2, 2, E], F16, kind="ExternalInput")
    d_id = nc.dram_tensor("ident", [128, 128], F16, kind="ExternalInput")
    d_y2 = nc.dram_tensor("y2", [BPC, E, N], F32, kind="ExternalOutput")

    def noldw(mm):
        (mm.ins if hasattr(mm, "ins") else mm).ldweights = False

    with nc.allow_low_precision(reason="fp16 matmul operand tiles"), \
         tile.TileContext(nc) as tc:
        with (
            tc.tile_pool(name="const", bufs=1) as const,
            tc.tile_pool(name="persist", bufs=1) as persist,
            tc.tile_pool(name="relp", bufs=1) as relp,
            tc.tile_pool(name="ee", bufs=6 + 2 * lka) as ee,
            tc.tile_pool(name="et", bufs=8 + 2 * lka) as et,
            tc.tile_pool(name="ps", bufs=2, space="PSUM") as ps,
            tc.tile_pool(name="po", bufs=1, space="PSUM") as po,
            tc.tile_pool(name="pw", bufs=1, space="PSUM") as pw,
            tc.tile_pool(name="pf", bufs=1, space="PSUM") as pf,
        ):
            # ---- constants ----
            wall_sb = const.tile([E, 4, E], F16, tag="wall")
            wk_sb = wall_sb[:, 0]
            wq_sb = wall_sb[:, 1]
            wv_sb = wall_sb[:, 2]
            wu_sb = wall_sb[:, 3]
            sel_sb = const.tile([2, 2, E], F16, tag="sel")
            id_sb = const.tile([128, 128], F16, tag="id")
            bq_sb = const.tile([E, 1], F32, tag="bq")
            bf_sb = const.tile([E, 1], F32, tag="bf")
            scr = const.tile([1, 2], F32, tag="scr")
            ln4_sb = const.tile([128, 1], F32, tag="ln4")
            nc.gpsimd.memset(ln4_sb[:], LN4)

            # PE warm-up burst: back-to-back dummy matmuls during the DMA
            # dead time flip the PE HAM clock gate to 2.4 GHz (~3.4us of
            # sustained activity) before the real pipeline starts; the
            # dense pipeline then sustains it.
            wuin = const.tile([128, 512], F16, tag="wuin")
            nc.gpsimd.memset(wuin[:], 0.0)
            pwu = pw.tile([128, 512], F32, tag="pw", name="pwu")

            def dummy_mm(cols=kdw):
                nc.tensor.matmul(pwu[:, 0:cols], wuin[:, 0:128],
                                 wuin[:, 0:cols], start=True, stop=True)

            for i in range(nwu):
                dummy_mm(512)

            x_sb, K_sb, Q_sb, VT_sb, out_sb, D_sb, bcr_sb, y_sb = (
                {}, {}, {}, {}, {}, {}, {}, {})
            for b in range(BPC):
                x_sb[b] = persist.tile([E, N], F16, tag=f"x{b}", name=f"x{b}")
                K_sb[b] = persist.tile([E, N], F16, tag=f"K{b}", name=f"K{b}")
                Q_sb[b] = persist.tile([E, N], F16, tag=f"Q{b}", name=f"Q{b}")
                # AV stationary per (t, pair, s): [128, 66]
                #   s=0: [V_h0 | 0 | 1 0],  s=1: [0 | V_h1 | 0 1]
                VT_sb[b] = persist.tile([128, NT, 2, 2, 66], F16, tag=f"VT{b}",
                                        name=f"VT{b}")
                out_sb[b] = persist.tile([E, N], F16, tag=f"O{b}", name=f"O{b}")
                D_sb[b] = {p: persist.tile([2, N], F16, tag=f"D{b}{p}",
                                           name=f"D{b}{p}") for p in range(2)}
                bcr_sb[b] = persist.tile([E, N], F32, tag=f"bcr{b}",
                                         name=f"bcr{b}")
                y_sb[b] = persist.tile([E, N], F32, tag=f"y{b}", name=f"y{b}")
            rel_t = {}
            for h in range(H):
                rel_t[h] = relp.tile([128, 64, 63], F16, tag=f"rel{h}",
                                     name=f"rel{h}")

            # ---- DMAs: x first on its own (scalar) queue so it lands as
            # early as possible; consts+rel FIFO on the sync queue ----
            for b in range(BPC):
                nc.scalar.dma_start(x_sb[b][:], d_x2.ap()[b])
            nc.sync.dma_start(wall_sb[:], d_wall.ap()[:])
            nc.sync.dma_start(sel_sb[:], d_sel.ap()[:])
            nc.sync.dma_start(id_sb[:], d_id.ap()[:])
            nc.sync.dma_start(bq_sb[:], d_bq.ap()[:])
            nc.sync.dma_start(bf_sb[:], d_bf.ap()[:])
            # rel on the same queue AFTER x/consts (FIFO keeps x first)
            for h in range(H):
                nc.sync.dma_start(
                    rel_t[h][:],
                    d_rel.ap()[h].rearrange("p (y x) -> p y x", x=63))

            # preload the ACT exp table while DMAs run
            nc.vector.memset(scr[:, 0:1], 0.0)
            nc.scalar.activation(scr[:, 1:2], scr[:, 0:1], AF.Exp)
            for b in range(BPC):
                nc.gpsimd.memset(VT_sb[b][:], 0.0)
                nc.gpsimd.memset(VT_sb[b][:, :, :, 0, 64:65], 1.0)
                nc.gpsimd.memset(VT_sb[b][:, :, :, 1, 65:66], 1.0)

            # ---- projections ----
            for b in range(BPC):
                pK = ps.tile([128, N], F32, tag="ps", name=f"pK{b}")
                for j in range(2):
                    js = ds(512 * j, 512)
                    mm = nc.tensor.matmul(pK[:, js], wk_sb, x_sb[b][:, js],
                                          start=True, stop=True)
                    if j == 1:
                        noldw(mm)
                nc.scalar.copy(K_sb[b][:], pK[:])
                pQ = ps.tile([128, N], F32, tag="ps", name=f"pQ{b}")
                for j in range(2):
                    js = ds(512 * j, 512)
                    mm = nc.tensor.matmul(pQ[:, js], wq_sb, x_sb[b][:, js],
                                          start=True, stop=True)
                    if j == 1:
                        noldw(mm)
                nc.vector.tensor_scalar_add(Q_sb[b][:], pQ[:], bq_sb[:])
                # V^T tiles: pV[:, t, p, s, c] = V[channel 64p+32s+c, key 128t+row]
                pV = ps.tile([128, NT, 2, 2, 32], F32, tag="ps", name=f"pV{b}")
                for t in range(NT):
                    nc.tensor.matmul(pV[:, t], x_sb[b][:, ts(t, 128)], wv_sb,
                                     start=True, stop=True)
                nc.vector.tensor_copy(VT_sb[b][:, :, :, 0, 0:32],
                                      pV[:, :, :, 0, :])
                nc.vector.tensor_copy(VT_sb[b][:, :, :, 1, 32:64],
                                      pV[:, :, :, 1, :])

            # ---- attention (units: (b, pair, t); AV lags one unit) ----
            units = [(b, p, t) for b in range(BPC) for p in range(2)
                     for t in range(NT)]
            po2 = {}
            pend = []        # deque of (b, p, t, {s: e_tile})
            uidx = 0

            def emit_av(b, p, t, ets):
                if t == 0:
                    po2[(b, p)] = po.tile([66, N], F32, tag="po",
                                          name=f"po{b}{p}")
                for s in range(2):
                    for j in range(2):
                        mm = nc.tensor.matmul(
                            po2[(b, p)][:, ds(512 * j, 512)],
                            VT_sb[b][:, t, p, s, :],
                            ets[s][:, ds(16 * j, 16), :],
                            start=(t == 0 and s == 0),
                            stop=(t == NT - 1 and s == 1),
                        )
                        if j == 1:
                            noldw(mm)

            def emit_epi_j(b, p, j):
                # evacuate one 512-col half of the head pair + denominators
                js = ds(512 * j, 512)
                nc.vector.tensor_copy(out_sb[b][ds(64 * p, 64), js],
                                      po2[(b, p)][0:64, js])
                nc.vector.tensor_copy(D_sb[b][p][:, js],
                                      po2[(b, p)][64:66, js])

            def emit_pair_epilogue(b, p):
                for j in range(2):
                    emit_epi_j(b, p, j)

            def emit_final_j(b, j):
                # divide + output projection, one 512-col phase on a
                # dedicated 1-bank PSUM pool (never blocks the att rotation)
                js = ds(512 * j, 512)
                pbc = pf.tile([128, 512], F32, tag="pf", name=f"pbc{b}{j}")
                for p in range(2):
                    nc.tensor.matmul(pbc[:], sel_sb[:, p],
                                     D_sb[b][p][:, js],
                                     start=(p == 0), stop=(p == 1))
                nc.vector.reciprocal_approx_fast(out=bcr_sb[b][:, js],
                                                 in_=pbc[:])
                nc.vector.tensor_mul(out_sb[b][:, js], out_sb[b][:, js],
                                     bcr_sb[b][:, js])
                py = pf.tile([128, 512], F32, tag="pf", name=f"py{b}{j}")
                nc.tensor.matmul(py[:], wu_sb, out_sb[b][:, js],
                                 start=True, stop=True)
                nc.vector.tensor_scalar_add(y_sb[b][:, js], py[:],
                                            bf_sb[:])
                nc.sync.dma_start(d_y2.ap()[b][:, js], y_sb[b][:, js])

            def emit_final(b):
                for j in range(2):
                    emit_final_j(b, j)

            final_q = []     # deferred emit_final, interleaved into the
            drained = 0      # next sample's unit stream (PE never stalls
            final_at = {}    # on the divide chain at sample boundaries)

            def drain_one():
                nonlocal drained
                pb, pp, pt, pets = pend.pop(0)
                emit_av(pb, pp, pt, pets)
                drained += 1
                if pt == NT - 1:
                    if pb == BPC - 1 and pp == 1:
                        # kernel tail: interleave per-j so the j0 divide
                        # chain overlaps the j1 AV matmuls and epilogue
                        for j in range(2):
                            emit_epi_j(pb, pp, j)
                            emit_final_j(pb, j)
                    else:
                        emit_pair_epilogue(pb, pp)
                        if pp == 1:
                            final_q.append(pb)
                            final_at[pb] = drained + 3
                while final_q and (final_at[final_q[0]] <= drained
                                   or not pend):
                    emit_final(final_q.pop(0))

            for b, p, t in units:
                # QK for both heads of the pair.  Head 2 (p==1, s==0) takes
                # the PE rel-add path: identity matmul accumulates the raw
                # Toeplitz rel window into att before QK.
                att = {}
                for s in range(2):
                    h = 2 * p + s
                    addpath = (h == 2)
                    att[s] = ps.tile([128, 32, 32], F32, tag="ps",
                                     name=f"att{s}")
                    relap3 = rel_t[h][:, ds(31 - 4 * t, 32), ds(31, 32)]
                    if addpath:
                        for j in range(2):
                            mm = nc.tensor.matmul(
                                att[s][:, ds(16 * j, 16), :],
                                id_sb[:],
                                rel_t[h][:, ds(31 - 4 * t + 16 * j, 16),
                                         ds(31, 32)],
                                start=True, stop=False,
                            )
                            if j == 1:
                                noldw(mm)
                    row = 64 * p + 32 * s
                    for j in range(2):
                        mm = nc.tensor.matmul(
                            att[s][:, ds(16 * j, 16), :],
                            K_sb[b][ds(row, 32), ts(t, 128)],
                            Q_sb[b][ds(row, 32), ds(512 * j, 512)],
                            start=not addpath, stop=True,
                            tile_position=(row, 0),
                        )
                        if j == 1:
                            noldw(mm)
                # elementwise: E = 4*exp(att) * exp_rel  (rel read through
                # the shared Toeplitz window: cols 1984-252t+63*y2+x2);
                # add-path tiles already hold rel inside att
                ets = {}
                for s in range(2):
                    h = 2 * p + s
                    addpath = (h == 2)
                    relap = rel_t[h][:, ds(31 - 4 * t, 32), ds(31, 32)]
                    if addpath:
                        # att already holds rel + ln4 from the table
                        e_t = et.tile([128, 32, 32], F16, tag="et", name="et")
                        nc.scalar.activation(e_t[:], att[s][:], AF.Exp)
                    elif uidx in sqset:
                        # (att/2 + 1)^2 ~= exp(att); the x4 lives in the table
                        u = ee.tile([128, 32, 32], F16, tag="ee", name="u")
                        nc.vector.tensor_scalar(u[:], att[s][:], 0.5, 1.0,
                                                ALU.mult, ALU.add)
                        sq = et.tile([128, 32, 32], F16, tag="et", name="sq")
                        nc.vector.tensor_mul(sq[:], u[:], u[:])
                        e_t = et.tile([128, 32, 32], F16, tag="et", name="et")
                        nc.vector.tensor_mul(e_t[:], sq[:], relap)
                    else:
                        ex = ee.tile([128, 32, 32], F16, tag="ee", name="ex")
                        nc.scalar.activation(ex[:], att[s][:], AF.Exp)
                        e_t = et.tile([128, 32, 32], F16, tag="et", name="et")
                        eng = nc.gpsimd if uidx in gpmul else nc.vector
                        eng.tensor_mul(e_t[:], ex[:], relap)
                    ets[s] = e_t
                    uidx += 1
                # AV lags `lka` units behind QK (software pipeline: PE
                # never waits on the elementwise chain)
                pend.append((b, p, t, ets))
                if len(pend) > lka:
                    drain_one()
                # warm-keeper: fill residual PE idle so the HAM clock gate
                # stays at 2.4 GHz (and self-heals if it ever drops);
                # rel-add units carry extra real PE work already
                for _ in range(ndm if p == 0 else max(0, ndm - 1)):
                    dummy_mm()
            while pend:
                drain_one()

    nc.compile()
    _CACHE[key] = nc
    return nc


def kernel(x, Wk, bk, Wq, bq, Wv, bv, Wu, bu, pos_enc):
    global LAST_RESULT
    x = np.ascontiguousarray(np.asarray(x, np.float32))
    Wk = np.asarray(Wk, np.float32)
    Wq = np.asarray(Wq, np.float32)
    Wv = np.asarray(Wv, np.float32)
    Wu = np.asarray(Wu, np.float32)
    bq = np.asarray(bq, np.float32)
    bv = np.asarray(bv, np.float32)
    bu = np.asarray(bu, np.float32)
    pos_enc = np.asarray(pos_enc, np.float32)

    wall = np.stack([Wk.T, (Wq * SCALE).T, Wv.T, Wu.T], axis=1)
    wall = np.ascontiguousarray(wall.astype(np.float16))
    bqv = np.ascontiguousarray((bq * SCALE).reshape(E, 1))
    bfv = np.ascontiguousarray((Wu @ bv + bu).reshape(E, 1))

    # compact Toeplitz table: relb[h, p, j] = exp(pos_enc)[h, S(p) + 3968 - j]
    # with S(p) = 63*(p//32) + p%32; kernel reads col 1984-252t+63*y2+x2
    # heads 0,1,3: exp flavor x4 (multiplied in);  head 2: raw flavor + ln4
    # (added into att by the PE identity matmul); the uniform factor 4
    # cancels in the softmax division and keeps exp() biases immediate
    tabP = 4.0 * np.exp(pos_enc)
    tabP[2] = pos_enc[2] + LN4
    fill = np.ones((H, 1, 1), np.float32)
    fill[2] = 0.0
    pidx = np.arange(128)
    S = (63 * (pidx // 32) + pidx % 32)[:, None]  # (128, 1)
    j = np.arange(4032)[None, :]                  # (1, 4032)
    tidx = S + 3968 - j                           # (128, 4032)
    valid = (tidx >= 0) & (tidx < 3969)
    relb = np.where(valid, tabP[:, tidx.clip(0, 3968)], fill)
    relb = np.ascontiguousarray(relb.astype(np.float16))
    ident = np.eye(128, dtype=np.float16)
    sel2 = np.zeros((2, 2, E), np.float16)
    for p in range(2):
        for s in range(2):
            sel2[s, p, 64 * p + 32 * s:64 * p + 32 * s + 32] = 1.0

    nc = _build()

    common = dict(wall=wall, bqv=bqv, bfv=bfv, relb=relb, sel2=sel2,
                  ident=ident)
    in_maps = []
    xr = x.reshape(B, E, N)
    for c in range(NCORES):
        m = dict(common)
        m["x2"] = np.ascontiguousarray(xr[BPC * c:BPC * (c + 1)].astype(np.float16))
        in_maps.append(m)

    trace = os.environ.get("BASS_TRACE", "") not in ("", "0")
    if trace:
        _ensure_ntff_hook()
    res = bass_utils.run_bass_kernel_spmd(
        nc, in_maps, core_ids=list(range(NCORES)), trace=trace)
    LAST_RESULT = res

    y = np.empty((B, E, N), np.float32)
    for c in range(NCORES):
        y[BPC * c:BPC * (c + 1)] = res.results[c]["y2"]
    return y.reshape(B, E, NY, NX)


# revision 77
# speedup vs baseline: 1.0398x; 1.0398x over previous
"""Attention2d Trainium2 kernel.

Reference computation (per sample b):
  K = Wk @ x;  Q = Wq @ x + bq;  V = Wv @ x + bv     (x: [128, 1024])
  per head h (32 channels):  att[k,q] = scale * K_h[:,k].Q_h[:,q] + rel_h[k,q]
  P = softmax_k(att);  out_h = V_h @ P;  y = Wu @ out + bu

Kernel strategy (8 NeuronCores, data-parallel over batch, 2 samples/core):
  - exp(att + rel) = exp(att) * exp(rel).  exp(rel) is gathered host-side
    (exp commutes with the pos_enc gather) and uploaded fp16; the multiply
    runs on the vector/gpsimd engines in SBUF.  This removes the rel-add
    identity matmuls from the PE entirely (1/3 of its matmul work).
  - exp(att) tiles are computed uniformly scaled by 4: ScalarE tiles use
    ACT Exp with bias=ln(4) -> 4*exp(att); a tunable subset of tiles runs
    on the vector engine as (att+2)^2 = 4*(1+att/2)^2 ~= 4*exp(att).  The
    scale cancels exactly in P = E/D, and the DVE path offloads the
    otherwise-bottleneck ScalarE.
  - AV matmul packs a head pair into one [66, N] PSUM accumulator via
    zero-padded stationaries [V_h0|0|ones|0] and [0|V_h1|0|ones]: channel
    rows 0..63 and both softmax denominators D at rows 64..65, so one copy
    evacuates the pair and the reciprocal reads D straight from PSUM.
  - division by D applied pre-Wu via a selector-matmul partition-broadcast
    of 1/D (softmax denominator), bv/bu folded into one final bias.
"""

import math
import os
import sys
import types

sys.path.insert(0, "/opt/trn_rl_repo")

import numpy as np

import concourse.bass as bass
import concourse.tile as tile
from concourse import bacc, mybir
from concourse import bass_utils
from concourse.bass import ds, ts

F32 = mybir.dt.float32
F16 = mybir.dt.float16
AF = mybir.ActivationFunctionType
ALU = mybir.AluOpType

B, E, H, NY, NX = 16, 128, 4, 32, 32
N = NY * NX          # 1024
HC = E // H          # 32
NCORES = 8
BPC = B // NCORES    # 2 samples per core
NT = N // 128        # 8 k-tiles
SCALE = HC ** -0.5
LN4 = math.log(4.0)

LAST_RESULT = None   # BassKernelResults of the most recent run (for test.py)

_CACHE = {}


def _patch_ldw_opt():
    """Enable walrus LDWEIGHTS elision (redundant identity reloads)."""
    if _CACHE.get("ldw_patched"):
        return
    import concourse.bass_utils as _bu
    orig = _bu.run_command

    def patched(argv, **kw):
        argv = [a.replace("--enable-ldw-opt=false", "--enable-ldw-opt=true")
                if isinstance(a, str) else a for a in argv]
        return orig(argv, **kw)

    _bu.run_command = patched
    _CACHE["ldw_patched"] = True


def _ensure_ntff_hook():
    """Register the axon NTFF profile hook that trn_boot couldn't install
    (the image lacks antenv.axon_hooks). Only needed when tracing."""
    if "antenv.axon_hooks" in sys.modules:
        return
    mod = types.ModuleType("antenv.axon_hooks")
    holder = [None]
    mod.set_axon_ntff_profile_hook = lambda h: holder.__setitem__(0, h)
    mod.get_axon_ntff_profile_hook = lambda: holder[0]
    sys.modules["antenv.axon_hooks"] = mod
    try:
        from trn_agent_boot.trn_boot import _ntff_profile_via_ctypes
        mod.set_axon_ntff_profile_hook(
            _ntff_profile_via_ctypes("/opt/axon/libaxon_pjrt.so")
        )
    except Exception:
        pass


def _rel_indices(ny, nx):
    y = np.arange(ny)
    x = np.arange(nx)
    y1, x1, y2, x2 = np.meshgrid(y, x, y, x, indexing="ij")
    idx = (y1 - y2 + ny - 1) * (2 * nx - 1) + (x1 - x2 + nx - 1)
    return idx.reshape(ny * nx, ny * nx)


def _spread(n, total=64):
    """Bresenham-spread set of n tile indices among `total` units."""
    out = set()
    if n <= 0:
        return out
    for i in range(total):
        if (i * n) // total != ((i + 1) * n) // total:
            out.add(i)
    return out


def _build():
    """Build + bacc-compile the per-core program (cached)."""
    if os.environ.get("KLDW", "0") == "1":
        _patch_ldw_opt()
    nsq = int(os.environ.get("KSQ", "0"))     # tiles on the DVE (x+2)^2 path
    ngp = int(os.environ.get("KMG", "8"))     # Sc-path rel-muls on GpSimd
    lka = int(os.environ.get("KLA", "5"))     # AV lookahead (units)
    nwu = int(os.environ.get("KWU", "36"))    # PE warm-up burst matmuls
    ndm = int(os.environ.get("KDM", "3"))     # warm-keeper matmuls per unit
    bfz = bool(int(os.environ.get("KBFZ", "1")))  # final bias known zero
    kdw = int(os.environ.get("KDW", "384"))   # warm-keeper matmul width
    key = ("nc", nsq, ngp, lka, nwu, ndm, bfz, kdw)
    if key in _CACHE:
        return _CACHE[key]
    _is_h2x = lambda i: ((i % 32) // 16 == 1) and (i % 2 == 0)
    sqset = {i for i in _spread(nsq) if not _is_h2x(i)}
    # GpSimd rel-muls: skip the first 12 tiles (6 units) so the pipe fills
    # without waiting on the slow engine; head-2 tiles are PE rel-add path
    _is_h2 = lambda i: ((i % 32) // 16 == 1) and (i % 2 == 0)
    scpath = [i for i in range(12, 64) if i not in sqset and not _is_h2(i)]
    gpmul = {scpath[i] for i in _spread(min(ngp, len(scpath)), len(scpath))}

    nc = bacc.Bacc("TRN2", target_bir_lowering=False, debug=False,
                   num_devices=NCORES)

    d_x2 = nc.dram_tensor("x2", [BPC, E, N], F16, kind="ExternalInput")
    d_wall = nc.dram_tensor("wall", [E, 4, E], F16, kind="ExternalInput")
    d_bq = nc.dram_tensor("bqv", [E, 1], F32, kind="ExternalInput")
    d_bf = nc.dram_tensor("bfv", [E, 1], F32, kind="ExternalInput")
    # compact Toeplitz rel table: T3[h, p, 1984-252t+63*y2+x2] =
    # exp(rel)[h, 128t+p, 32*y2+x2]  (one shared table for all key-tiles t)
    d_rel = nc.dram_tensor("relb", [H, 128, 4032], F16, kind="ExternalInput")
    d_sel = nc.dram_tensor("sel2", [2, 2, E], F16, kind="ExternalInput")
    d_id = nc.dram_tensor("ident", [128, 128], F16, kind="ExternalInput")
    d_y2 = nc.dram_tensor("y2", [BPC, E, N], F32, kind="ExternalOutput")

    def noldw(mm):
        (mm.ins if hasattr(mm, "ins") else mm).ldweights = False

    with nc.allow_low_precision(reason="fp16 matmul operand tiles"), \
         tile.TileContext(nc) as tc:
        with (
            tc.tile_pool(name="const", bufs=1) as const,
            tc.tile_pool(name="persist", bufs=1) as persist,
            tc.tile_pool(name="relp", bufs=1) as relp,
            tc.tile_pool(name="ee", bufs=6 + 2 * lka) as ee,
            tc.tile_pool(name="et", bufs=8 + 2 * lka) as et,
            tc.tile_pool(name="ps", bufs=2, space="PSUM") as ps,
            tc.tile_pool(name="po", bufs=2, space="PSUM") as po,
            tc.tile_pool(name="pw", bufs=1, space="PSUM") as pw,
            tc.tile_pool(name="pf", bufs=1, space="PSUM") as pf,
        ):
            # ---- constants ----
            wall_sb = const.tile([E, 4, E], F16, tag="wall")
            wk_sb = wall_sb[:, 0]
            wq_sb = wall_sb[:, 1]
            wv_sb = wall_sb[:, 2]
            wu_sb = wall_sb[:, 3]
            sel_sb = const.tile([2, 2, E], F16, tag="sel")
            id_sb = const.tile([128, 128], F16, tag="id")
            bq_sb = const.tile([E, 1], F32, tag="bq")
            bf_sb = const.tile([E, 1], F32, tag="bf")
            scr = const.tile([1, 2], F32, tag="scr")
            ln4_sb = const.tile([128, 1], F32, tag="ln4")
            nc.gpsimd.memset(ln4_sb[:], LN4)

            # PE warm-up burst: back-to-back dummy matmuls during the DMA
            # dead time flip the PE HAM clock gate to 2.4 GHz (~3.4us of
            # sustained activity) before the real pipeline starts; the
            # dense pipeline then sustains it.
            wuin = const.tile([128, 512], F16, tag="wuin")
            nc.gpsimd.memset(wuin[:], 0.0)
            pwu = pw.tile([128, 512], F32, tag="pw", name="pwu")

            def dummy_mm(cols=kdw):
                nc.tensor.matmul(pwu[:, 0:cols], wuin[:, 0:128],
                                 wuin[:, 0:cols], start=True, stop=True)

            for i in range(nwu):
                dummy_mm(512)

            x_sb, K_sb, Q_sb, VT_sb, out_sb, D_sb, bcr_sb, y_sb = (
                {}, {}, {}, {}, {}, {}, {}, {})
            for b in range(BPC):
                x_sb[b] = persist.tile([E, N], F16, tag=f"x{b}", name=f"x{b}")
                K_sb[b] = persist.tile([E, N], F16, tag=f"K{b}", name=f"K{b}")
                Q_sb[b] = persist.tile([E, N], F16, tag=f"Q{b}", name=f"Q{b}")
                # AV stationary per (t, pair, s): [128, 66]
                #   s=0: [V_h0 | 0 | 1 0],  s=1: [0 | V_h1 | 0 1]
                VT_sb[b] = persist.tile([128, NT, 2, 2, 66], F16, tag=f"VT{b}",
                                        name=f"VT{b}")
                out_sb[b] = persist.tile([E, N], F16, tag=f"O{b}", name=f"O{b}")
                D_sb[b] = {p: persist.tile([2, N], F16, tag=f"D{b}{p}",
                                           name=f"D{b}{p}") for p in range(2)}
                bcr_sb[b] = persist.tile([E, N], F32, tag=f"bcr{b}",
                                         name=f"bcr{b}")
                y_sb[b] = persist.tile([E, N], F32, tag=f"y{b}", name=f"y{b}")
            rel_t = {}
            for h in range(H):
                rel_t[h] = relp.tile([128, 64, 63], F16, tag=f"rel{h}",
                                     name=f"rel{h}")

            # ---- DMAs: x first on its own (scalar) queue so it lands as
            # early as possible; consts+rel FIFO on the sync queue ----
            for b in range(BPC):
                nc.scalar.dma_start(x_sb[b][:], d_x2.ap()[b])
            nc.sync.dma_start(wall_sb[:], d_wall.ap()[:])
            nc.sync.dma_start(sel_sb[:], d_sel.ap()[:])
            nc.sync.dma_start(id_sb[:], d_id.ap()[:])
            nc.sync.dma_start(bq_sb[:], d_bq.ap()[:])
            nc.sync.dma_start(bf_sb[:], d_bf.ap()[:])
            # rel on the same queue AFTER x/consts (FIFO keeps x first)
            for h in range(H):
                nc.sync.dma_start(
                    rel_t[h][:],
                    d_rel.ap()[h].rearrange("p (y x) -> p y x", x=63))

            # preload the ACT exp table while DMAs run
            nc.vector.memset(scr[:, 0:1], 0.0)
            nc.scalar.activation(scr[:, 1:2], scr[:, 0:1], AF.Exp)
            for b in range(BPC):
                nc.gpsimd.memset(VT_sb[b][:], 0.0)
                nc.gpsimd.memset(VT_sb[b][:, :, :, 0, 64:65], 1.0)
                nc.gpsimd.memset(VT_sb[b][:, :, :, 1, 65:66], 1.0)

            # ---- projections ----
            for b in range(BPC):
                pK = ps.tile([128, N], F32, tag="ps", name=f"pK{b}")
                for j in range(2):
                    js = ds(512 * j, 512)
                    mm = nc.tensor.matmul(pK[:, js], wk_sb, x_sb[b][:, js],
                                          start=True, stop=True)
                    if j == 1:
                        noldw(mm)
                nc.scalar.copy(K_sb[b][:], pK[:])
                pQ = ps.tile([128, N], F32, tag="ps", name=f"pQ{b}")
                for j in range(2):
                    js = ds(512 * j, 512)
                    mm = nc.tensor.matmul(pQ[:, js], wq_sb, x_sb[b][:, js],
                                          start=True, stop=True)
                    if j == 1:
                        noldw(mm)
                nc.vector.tensor_scalar_add(Q_sb[b][:], pQ[:], bq_sb[:])
                # V^T tiles: pV[:, t, p, s, c] = V[channel 64p+32s+c, key 128t+row]
                pV = ps.tile([128, NT, 2, 2, 32], F32, tag="ps", name=f"pV{b}")
                for t in range(NT):
                    nc.tensor.matmul(pV[:, t], x_sb[b][:, ts(t, 128)], wv_sb,
                                     start=True, stop=True)
                nc.vector.tensor_copy(VT_sb[b][:, :, :, 0, 0:32],
                                      pV[:, :, :, 0, :])
                nc.vector.tensor_copy(VT_sb[b][:, :, :, 1, 32:64],
                                      pV[:, :, :, 1, :])

            # ---- attention (units: (b, pair, t); AV lags one unit) ----
            units = [(b, p, t) for b in range(BPC) for p in range(2)
                     for t in range(NT)]
            po2 = {}
            pend = []        # deque of (b, p, t, {s: e_tile})
            uidx = 0

            def emit_av(b, p, t, ets):
                if t == 0:
                    # two per-j accumulators (1 bank each) so the j0 half
                    # is dependency-complete before the j1 stream finishes
                    po2[(b, p)] = [po.tile([66, 512], F32, tag="po",
                                           name=f"po{b}{p}{j}")
                                   for j in range(2)]
                for s in range(2):
                    for j in range(2):
                        mm = nc.tensor.matmul(
                            po2[(b, p)][j][:],
                            VT_sb[b][:, t, p, s, :],
                            ets[s][:, ds(16 * j, 16), :],
                            start=(t == 0 and s == 0),
                            stop=(t == NT - 1 and s == 1),
                        )
                        if j == 1:
                            noldw(mm)

            def emit_epi_j(b, p, j):
                # evacuate one 512-col half of the head pair + denominators
                js = ds(512 * j, 512)
                nc.vector.tensor_copy(out_sb[b][ds(64 * p, 64), js],
                                      po2[(b, p)][j][0:64, :])
                nc.vector.tensor_copy(D_sb[b][p][:, js],
                                      po2[(b, p)][j][64:66, :])

            def emit_pair_epilogue(b, p):
                for j in range(2):
                    emit_epi_j(b, p, j)

            def emit_final_j(b, j):
                # divide + output projection, one 512-col phase on a
                # dedicated 1-bank PSUM pool (never blocks the att rotation)
                js = ds(512 * j, 512)
                pbc = pf.tile([128, 512], F32, tag="pf", name=f"pbc{b}{j}")
                for p in range(2):
                    nc.tensor.matmul(pbc[:], sel_sb[:, p],
                                     D_sb[b][p][:, js],
                                     start=(p == 0), stop=(p == 1))
                nc.vector.reciprocal_approx_fast(out=bcr_sb[b][:, js],
                                                 in_=pbc[:])
                nc.vector.tensor_mul(out_sb[b][:, js], out_sb[b][:, js],
                                     bcr_sb[b][:, js])
                py = pf.tile([128, 512], F32, tag="pf", name=f"py{b}{j}")
                nc.tensor.matmul(py[:], wu_sb, out_sb[b][:, js],
                                 start=True, stop=True)
                nc.vector.tensor_scalar_add(y_sb[b][:, js], py[:],
                                            bf_sb[:])
                nc.sync.dma_start(d_y2.ap()[b][:, js], y_sb[b][:, js])

            def emit_final(b):
                for j in range(2):
                    emit_final_j(b, j)

            final_q = []     # deferred emit_final, interleaved into the
            drained = 0      # next sample's unit stream (PE never stalls
            final_at = {}    # on the divide chain at sample boundaries)

            def drain_one():
                nonlocal drained
                pb, pp, pt, pets = pend.pop(0)
                emit_av(pb, pp, pt, pets)
                drained += 1
                if pt == NT - 1:
                    if pb == BPC - 1 and pp == 1:
                        # kernel tail: interleave per-j so the j0 divide
                        # chain overlaps the j1 AV matmuls and epilogue
                        for j in range(2):
                            emit_epi_j(pb, pp, j)
                            emit_final_j(pb, j)
                    else:
                        emit_pair_epilogue(pb, pp)
                        if pp == 1:
                            final_q.append(pb)
                            final_at[pb] = drained + 3
                while final_q and (final_at[final_q[0]] <= drained
                                   or not pend):
                    emit_final(final_q.pop(0))

            for b, p, t in units:
                # QK for both heads of the pair.  Head 2 (p==1, s==0) takes
                # the PE rel-add path: identity matmul accumulates the raw
                # Toeplitz rel window into att before QK.
                att = {}
                for s in range(2):
                    h = 2 * p + s
                    addpath = (h == 2)
                    att[s] = ps.tile([128, 32, 32], F32, tag="ps",
                                     name=f"att{s}")
                    relap3 = rel_t[h][:, ds(31 - 4 * t, 32), ds(31, 32)]
                    if addpath:
                        for j in range(2):
                            mm = nc.tensor.matmul(
                                att[s][:, ds(16 * j, 16), :],
                                id_sb[:],
                                rel_t[h][:, ds(31 - 4 * t + 16 * j, 16),
                                         ds(31, 32)],
                                start=True, stop=False,
                            )
                            if j == 1:
                                noldw(mm)
                    row = 64 * p + 32 * s
                    for j in range(2):
                        mm = nc.tensor.matmul(
                            att[s][:, ds(16 * j, 16), :],
                            K_sb[b][ds(row, 32), ts(t, 128)],
                            Q_sb[b][ds(row, 32), ds(512 * j, 512)],
                            start=not addpath, stop=True,
                            tile_position=(row, 0),
                        )
                        if j == 1:
                            noldw(mm)
                # elementwise: E = 4*exp(att) * exp_rel  (rel read through
                # the shared Toeplitz window: cols 1984-252t+63*y2+x2);
                # add-path tiles already hold rel inside att
                ets = {}
                for s in range(2):
                    h = 2 * p + s
                    addpath = (h == 2)
                    relap = rel_t[h][:, ds(31 - 4 * t, 32), ds(31, 32)]
                    if addpath:
                        # att already holds rel + ln4 from the table
                        e_t = et.tile([128, 32, 32], F16, tag="et", name="et")
                        nc.scalar.activation(e_t[:], att[s][:], AF.Exp)
                    elif uidx in sqset:
                        # (att/2 + 1)^2 ~= exp(att); the x4 lives in the table
                        u = ee.tile([128, 32, 32], F16, tag="ee", name="u")
                        nc.vector.tensor_scalar(u[:], att[s][:], 0.5, 1.0,
                                                ALU.mult, ALU.add)
                        sq = et.tile([128, 32, 32], F16, tag="et", name="sq")
                        nc.vector.tensor_mul(sq[:], u[:], u[:])
                        e_t = et.tile([128, 32, 32], F16, tag="et", name="et")
                        nc.vector.tensor_mul(e_t[:], sq[:], relap)
                    else:
                        ex = ee.tile([128, 32, 32], F16, tag="ee", name="ex")
                        nc.scalar.activation(ex[:], att[s][:], AF.Exp)
                        e_t = et.tile([128, 32, 32], F16, tag="et", name="et")
                        eng = nc.gpsimd if uidx in gpmul else nc.vector
                        eng.tensor_mul(e_t[:], ex[:], relap)
                    ets[s] = e_t
                    uidx += 1
                # AV lags `lka` units behind QK (software pipeline: PE
                # never waits on the elementwise chain)
                pend.append((b, p, t, ets))
                if len(pend) > lka:
                    drain_one()
                # warm-keeper: fill residual PE idle so the HAM clock gate
                # stays at 2.4 GHz (and self-heals if it ever drops);
                # rel-add units carry extra real PE work already
                for _ in range(ndm if p == 0 else max(0, ndm - 1)):
                    dummy_mm()
            while pend:
                drain_one()

    nc.compile()
    _CACHE[key] = nc
    return nc


def kernel(x, Wk, bk, Wq, bq, Wv, bv, Wu, bu, pos_enc):
    global LAST_RESULT
    x = np.ascontiguousarray(np.asarray(x, np.float32))
    Wk = np.asarray(Wk, np.float32)
    Wq = np.asarray(Wq, np.float32)
    Wv = np.asarray(Wv, np.float32)
    Wu = np.asarray(Wu, np.float32)
    bq = np.asarray(bq, np.float32)
    bv = np.asarray(bv, np.float32)
    bu = np.asarray(bu, np.float32)
    pos_enc = np.asarray(pos_enc, np.float32)

    wall = np.stack([Wk.T, (Wq * SCALE).T, Wv.T, Wu.T], axis=1)
    wall = np.ascontiguousarray(wall.astype(np.float16))
    bqv = np.ascontiguousarray((bq * SCALE).reshape(E, 1))
    bfv = np.ascontiguousarray((Wu @ bv + bu).reshape(E, 1))

    # compact Toeplitz table: relb[h, p, j] = exp(pos_enc)[h, S(p) + 3968 - j]
    # with S(p) = 63*(p//32) + p%32; kernel reads col 1984-252t+63*y2+x2
    # heads 0,1,3: exp flavor x4 (multiplied in);  head 2: raw flavor + ln4
    # (added into att by the PE identity matmul); the uniform factor 4
    # cancels in the softmax division and keeps exp() biases immediate
    tabP = 4.0 * np.exp(pos_enc)
    tabP[2] = pos_enc[2] + LN4
    fill = np.ones((H, 1, 1), np.float32)
    fill[2] = 0.0
    pidx = np.arange(128)
    S = (63 * (pidx // 32) + pidx % 32)[:, None]  # (128, 1)
    j = np.arange(4032)[None, :]                  # (1, 4032)
    tidx = S + 3968 - j                           # (128, 4032)
    valid = (tidx >= 0) & (tidx < 3969)
    relb = np.where(valid, tabP[:, tidx.clip(0, 3968)], fill)
    relb = np.ascontiguousarray(relb.astype(np.float16))
    ident = np.eye(128, dtype=np.float16)
    sel2 = np.zeros((2, 2, E), np.float16)
    for p in range(2):
        for s in range(2):
            sel2[s, p, 64 * p + 32 * s:64 * p + 32 * s + 32] = 1.0

    nc = _build()

    common = dict(wall=wall, bqv=bqv, bfv=bfv, relb=relb, sel2=sel2,
                  ident=ident)
    in_maps = []
    xr = x.reshape(B, E, N)
    for c in range(NCORES):
        m = dict(common)
        m["x2"] = np.ascontiguousarray(xr[BPC * c:BPC * (c + 1)].astype(np.float16))
        in_maps.append(m)

    trace = os.environ.get("BASS_TRACE", "") not in ("", "0")
    if trace:
        _ensure_ntff_hook()
    res = bass_utils.run_bass_kernel_spmd(
        nc, in_maps, core_ids=list(range(NCORES)), trace=trace)
    LAST_RESULT = res

    y = np.empty((B, E, N), np.float32)
    for c in range(NCORES):
        y[BPC * c:BPC * (c + 1)] = res.results[c]["y2"]
    return y.reshape(B, E, NY, NX)


# revision 79
# speedup vs baseline: 1.0413x; 1.0015x over previous
"""Attention2d Trainium2 kernel.

Reference computation (per sample b):
  K = Wk @ x;  Q = Wq @ x + bq;  V = Wv @ x + bv     (x: [128, 1024])
  per head h (32 channels):  att[k,q] = scale * K_h[:,k].Q_h[:,q] + rel_h[k,q]
  P = softmax_k(att);  out_h = V_h @ P;  y = Wu @ out + bu

Kernel strategy (8 NeuronCores, data-parallel over batch, 2 samples/core):
  - exp(att + rel) = exp(att) * exp(rel).  exp(rel) is gathered host-side
    (exp commutes with the pos_enc gather) and uploaded fp16; the multiply
    runs on the vector/gpsimd engines in SBUF.  This removes the rel-add
    identity matmuls from the PE entirely (1/3 of its matmul work).
  - exp(att) tiles are computed uniformly scaled by 4: ScalarE tiles use
    ACT Exp with bias=ln(4) -> 4*exp(att); a tunable subset of tiles runs
    on the vector engine as (att+2)^2 = 4*(1+att/2)^2 ~= 4*exp(att).  The
    scale cancels exactly in P = E/D, and the DVE path offloads the
    otherwise-bottleneck ScalarE.
  - AV matmul packs a head pair into one [66, N] PSUM accumulator via
    zero-padded stationaries [V_h0|0|ones|0] and [0|V_h1|0|ones]: channel
    rows 0..63 and both softmax denominators D at rows 64..65, so one copy
    evacuates the pair and the reciprocal reads D straight from PSUM.
  - division by D applied pre-Wu via a selector-matmul partition-broadcast
    of 1/D (softmax denominator), bv/bu folded into one final bias.
"""

import math
import os
import sys
import types

sys.path.insert(0, "/opt/trn_rl_repo")

import numpy as np

import concourse.bass as bass
import concourse.tile as tile
from concourse import bacc, mybir
from concourse import bass_utils
from concourse.bass import ds, ts

F32 = mybir.dt.float32
F16 = mybir.dt.float16
AF = mybir.ActivationFunctionType
ALU = mybir.AluOpType

B, E, H, NY, NX = 16, 128, 4, 32, 32
N = NY * NX          # 1024
HC = E // H          # 32
NCORES = 8
BPC = B // NCORES    # 2 samples per core
NT = N // 128        # 8 k-tiles
SCALE = HC ** -0.5
LN4 = math.log(4.0)

LAST_RESULT = None   # BassKernelResults of the most recent run (for test.py)

_CACHE = {}


def _patch_ldw_opt():
    """Enable walrus LDWEIGHTS elision (redundant identity reloads)."""
    if _CACHE.get("ldw_patched"):
        return
    import concourse.bass_utils as _bu
    orig = _bu.run_command

    def patched(argv, **kw):
        argv = [a.replace("--enable-ldw-opt=false", "--enable-ldw-opt=true")
                if isinstance(a, str) else a for a in argv]
        return orig(argv, **kw)

    _bu.run_command = patched
    _CACHE["ldw_patched"] = True


def _ensure_ntff_hook():
    """Register the axon NTFF profile hook that trn_boot couldn't install
    (the image lacks antenv.axon_hooks). Only needed when tracing."""
    if "antenv.axon_hooks" in sys.modules:
        return
    mod = types.ModuleType("antenv.axon_hooks")
    holder = [None]
    mod.set_axon_ntff_profile_hook = lambda h: holder.__setitem__(0, h)
    mod.get_axon_ntff_profile_hook = lambda: holder[0]
    sys.modules["antenv.axon_hooks"] = mod
    try:
        from trn_agent_boot.trn_boot import _ntff_profile_via_ctypes
        mod.set_axon_ntff_profile_hook(
            _ntff_profile_via_ctypes("/opt/axon/libaxon_pjrt.so")
        )
    except Exception:
        pass


def _rel_indices(ny, nx):
    y = np.arange(ny)
    x = np.arange(nx)
    y1, x1, y2, x2 = np.meshgrid(y, x, y, x, indexing="ij")
    idx = (y1 - y2 + ny - 1) * (2 * nx - 1) + (x1 - x2 + nx - 1)
    return idx.reshape(ny * nx, ny * nx)


def _spread(n, total=64):
    """Bresenham-spread set of n tile indices among `total` units."""
    out = set()
    if n <= 0:
        return out
    for i in range(total):
        if (i * n) // total != ((i + 1) * n) // total:
            out.add(i)
    return out


def _build():
    """Build + bacc-compile the per-core program (cached)."""
    if os.environ.get("KLDW", "0") == "1":
        _patch_ldw_opt()
    nsq = int(os.environ.get("KSQ", "0"))     # tiles on the DVE (x+2)^2 path
    ngp = int(os.environ.get("KMG", "8"))     # Sc-path rel-muls on GpSimd
    lka = int(os.environ.get("KLA", "5"))     # AV lookahead (units)
    nwu = int(os.environ.get("KWU", "36"))    # PE warm-up burst matmuls
    ndm = int(os.environ.get("KDM", "3"))     # warm-keeper matmuls per unit
    bfz = bool(int(os.environ.get("KBFZ", "1")))  # final bias known zero
    kdw = int(os.environ.get("KDW", "384"))   # warm-keeper matmul width
    key = ("nc", nsq, ngp, lka, nwu, ndm, bfz, kdw)
    if key in _CACHE:
        return _CACHE[key]
    _is_h2x = lambda i: ((i % 32) // 16 == 1) and (i % 2 == 0)
    sqset = {i for i in _spread(nsq) if not _is_h2x(i)}
    # GpSimd rel-muls: skip the first 12 tiles (6 units) so the pipe fills
    # without waiting on the slow engine; head-2 tiles are PE rel-add path
    _is_h2 = lambda i: ((i % 32) // 16 == 1) and (i % 2 == 0)
    scpath = [i for i in range(12, 64) if i not in sqset and not _is_h2(i)]
    gpmul = {scpath[i] for i in _spread(min(ngp, len(scpath)), len(scpath))}

    nc = bacc.Bacc("TRN2", target_bir_lowering=False, debug=False,
                   num_devices=NCORES)

    d_x2 = nc.dram_tensor("x2", [BPC, E, N], F16, kind="ExternalInput")
    d_wall = nc.dram_tensor("wall", [E, 4, E], F16, kind="ExternalInput")
    d_bq = nc.dram_tensor("bqv", [E, 1], F32, kind="ExternalInput")
    d_bf = nc.dram_tensor("bfv", [E, 1], F32, kind="ExternalInput")
    # compact Toeplitz rel table: T3[h, p, 1984-252t+63*y2+x2] =
    # exp(rel)[h, 128t+p, 32*y2+x2]  (one shared table for all key-tiles t)
    d_rel = nc.dram_tensor("relb", [H, 128, 4032], F16, kind="ExternalInput")
    d_sel = nc.dram_tensor("sel2", [2, 2, E], F16, kind="ExternalInput")
    d_id = nc.dram_tensor("ident", [128, 128], F16, kind="ExternalInput")
    d_y2 = nc.dram_tensor("y2", [BPC, E, N], F32, kind="ExternalOutput")

    def noldw(mm):
        (mm.ins if hasattr(mm, "ins") else mm).ldweights = False

    with nc.allow_low_precision(reason="fp16 matmul operand tiles"), \
         tile.TileContext(nc) as tc:
        with (
            tc.tile_pool(name="const", bufs=1) as const,
            tc.tile_pool(name="persist", bufs=1) as persist,
            tc.tile_pool(name="relp", bufs=1) as relp,
            tc.tile_pool(name="ee", bufs=6 + 2 * lka) as ee,
            tc.tile_pool(name="et", bufs=8 + 2 * lka) as et,
            tc.tile_pool(name="ps", bufs=2, space="PSUM") as ps,
            tc.tile_pool(name="po", bufs=2, space="PSUM") as po,
            tc.tile_pool(name="pw", bufs=1, space="PSUM") as pw,
            tc.tile_pool(name="pf", bufs=1, space="PSUM") as pf,
        ):
            # ---- constants ----
            wall_sb = const.tile([E, 4, E], F16, tag="wall")
            wk_sb = wall_sb[:, 0]
            wq_sb = wall_sb[:, 1]
            wv_sb = wall_sb[:, 2]
            wu_sb = wall_sb[:, 3]
            sel_sb = const.tile([2, 2, E], F16, tag="sel")
            id_sb = const.tile([128, 128], F16, tag="id")
            bq_sb = const.tile([E, 1], F32, tag="bq")
            bf_sb = const.tile([E, 1], F32, tag="bf")
            scr = const.tile([1, 2], F32, tag="scr")
            ln4_sb = const.tile([128, 1], F32, tag="ln4")
            nc.gpsimd.memset(ln4_sb[:], LN4)

            # PE warm-up burst: back-to-back dummy matmuls during the DMA
            # dead time flip the PE HAM clock gate to 2.4 GHz (~3.4us of
            # sustained activity) before the real pipeline starts; the
            # dense pipeline then sustains it.
            wuin = const.tile([128, 512], F16, tag="wuin")
            nc.gpsimd.memset(wuin[:], 0.0)
            pwu = pw.tile([128, 512], F32, tag="pw", name="pwu")

            def dummy_mm(cols=kdw):
                nc.tensor.matmul(pwu[:, 0:cols], wuin[:, 0:128],
                                 wuin[:, 0:cols], start=True, stop=True)

            for i in range(nwu):
                dummy_mm(512)

            x_sb, K_sb, Q_sb, VT_sb, out_sb, D_sb, bcr_sb, y_sb = (
                {}, {}, {}, {}, {}, {}, {}, {})
            for b in range(BPC):
                x_sb[b] = persist.tile([E, N], F16, tag=f"x{b}", name=f"x{b}")
                K_sb[b] = persist.tile([E, N], F16, tag=f"K{b}", name=f"K{b}")
                Q_sb[b] = persist.tile([E, N], F16, tag=f"Q{b}", name=f"Q{b}")
                # AV stationary per (t, pair, s): [128, 66]
                #   s=0: [V_h0 | 0 | 1 0],  s=1: [0 | V_h1 | 0 1]
                VT_sb[b] = persist.tile([128, NT, 2, 2, 66], F16, tag=f"VT{b}",
                                        name=f"VT{b}")
                out_sb[b] = persist.tile([E, N], F16, tag=f"O{b}", name=f"O{b}")
                D_sb[b] = {p: persist.tile([2, N], F16, tag=f"D{b}{p}",
                                           name=f"D{b}{p}") for p in range(2)}
                bcr_sb[b] = persist.tile([E, N], F32, tag=f"bcr{b}",
                                         name=f"bcr{b}")
                y_sb[b] = persist.tile([E, N], F32, tag=f"y{b}", name=f"y{b}")
            rel_t = {}
            for h in range(H):
                rel_t[h] = relp.tile([128, 64, 63], F16, tag=f"rel{h}",
                                     name=f"rel{h}")

            # ---- DMAs: x first on its own (scalar) queue so it lands as
            # early as possible; consts+rel FIFO on the sync queue ----
            for b in range(BPC):
                nc.scalar.dma_start(x_sb[b][:], d_x2.ap()[b])
            nc.sync.dma_start(wall_sb[:], d_wall.ap()[:])
            nc.sync.dma_start(sel_sb[:], d_sel.ap()[:])
            nc.sync.dma_start(id_sb[:], d_id.ap()[:])
            nc.sync.dma_start(bq_sb[:], d_bq.ap()[:])
            nc.sync.dma_start(bf_sb[:], d_bf.ap()[:])
            # rel on the same queue AFTER x/consts (FIFO keeps x first)
            for h in range(H):
                nc.sync.dma_start(
                    rel_t[h][:],
                    d_rel.ap()[h].rearrange("p (y x) -> p y x", x=63))

            # preload the ACT exp table while DMAs run
            nc.vector.memset(scr[:, 0:1], 0.0)
            nc.scalar.activation(scr[:, 1:2], scr[:, 0:1], AF.Exp)
            for b in range(BPC):
                nc.gpsimd.memset(VT_sb[b][:], 0.0)
                nc.gpsimd.memset(VT_sb[b][:, :, :, 0, 64:65], 1.0)
                nc.gpsimd.memset(VT_sb[b][:, :, :, 1, 65:66], 1.0)

            # ---- projections ----
            for b in range(BPC):
                pK = ps.tile([128, N], F32, tag="ps", name=f"pK{b}")
                for j in range(2):
                    js = ds(512 * j, 512)
                    mm = nc.tensor.matmul(pK[:, js], wk_sb, x_sb[b][:, js],
                                          start=True, stop=True)
                    if j == 1:
                        noldw(mm)
                nc.scalar.copy(K_sb[b][:], pK[:])
                pQ = ps.tile([128, N], F32, tag="ps", name=f"pQ{b}")
                for j in range(2):
                    js = ds(512 * j, 512)
                    mm = nc.tensor.matmul(pQ[:, js], wq_sb, x_sb[b][:, js],
                                          start=True, stop=True)
                    if j == 1:
                        noldw(mm)
                nc.vector.tensor_scalar_add(Q_sb[b][:], pQ[:], bq_sb[:])
                # V^T tiles: pV[:, t, p, s, c] = V[channel 64p+32s+c, key 128t+row]
                pV = ps.tile([128, NT, 2, 2, 32], F32, tag="ps", name=f"pV{b}")
                for t in range(NT):
                    nc.tensor.matmul(pV[:, t], x_sb[b][:, ts(t, 128)], wv_sb,
                                     start=True, stop=True)
                nc.vector.tensor_copy(VT_sb[b][:, :, :, 0, 0:32],
                                      pV[:, :, :, 0, :])
                nc.vector.tensor_copy(VT_sb[b][:, :, :, 1, 32:64],
                                      pV[:, :, :, 1, :])

            # ---- attention (units: (b, pair, t); AV lags one unit) ----
            units = [(b, p, t) for b in range(BPC) for p in range(2)
                     for t in range(NT)]
            po2 = {}
            pend = []        # deque of (b, p, t, {s: e_tile})
            uidx = 0

            def emit_av(b, p, t, ets):
                if t == 0:
                    # two per-j accumulators (1 bank each) so the j0 half
                    # is dependency-complete before the j1 stream finishes
                    po2[(b, p)] = [po.tile([66, 512], F32, tag="po",
                                           name=f"po{b}{p}{j}")
                                   for j in range(2)]
                for s in range(2):
                    for j in range(2):
                        mm = nc.tensor.matmul(
                            po2[(b, p)][j][:],
                            VT_sb[b][:, t, p, s, :],
                            ets[s][:, ds(16 * j, 16), :],
                            start=(t == 0 and s == 0),
                            stop=(t == NT - 1 and s == 1),
                        )
                        if j == 1:
                            noldw(mm)

            def emit_epi_j(b, p, j, tail=False):
                # evacuate one 512-col half of the head pair + denominators;
                # at the kernel tail use the (then-idle) scalar engine so
                # these never queue behind the DVE divide chain
                js = ds(512 * j, 512)
                if tail:
                    nc.scalar.copy(out_sb[b][ds(64 * p, 64), js],
                                   po2[(b, p)][j][0:64, :])
                    nc.scalar.copy(D_sb[b][p][:, js],
                                   po2[(b, p)][j][64:66, :])
                else:
                    nc.vector.tensor_copy(out_sb[b][ds(64 * p, 64), js],
                                          po2[(b, p)][j][0:64, :])
                    nc.vector.tensor_copy(D_sb[b][p][:, js],
                                          po2[(b, p)][j][64:66, :])

            def emit_pair_epilogue(b, p):
                for j in range(2):
                    emit_epi_j(b, p, j)

            def emit_final_j(b, j):
                # divide + output projection, one 512-col phase on a
                # dedicated 1-bank PSUM pool (never blocks the att rotation)
                js = ds(512 * j, 512)
                pbc = pf.tile([128, 512], F32, tag="pf", name=f"pbc{b}{j}")
                for p in range(2):
                    nc.tensor.matmul(pbc[:], sel_sb[:, p],
                                     D_sb[b][p][:, js],
                                     start=(p == 0), stop=(p == 1))
                nc.vector.reciprocal_approx_fast(out=bcr_sb[b][:, js],
                                                 in_=pbc[:])
                nc.vector.tensor_mul(out_sb[b][:, js], out_sb[b][:, js],
                                     bcr_sb[b][:, js])
                py = pf.tile([128, 512], F32, tag="pf", name=f"py{b}{j}")
                nc.tensor.matmul(py[:], wu_sb, out_sb[b][:, js],
                                 start=True, stop=True)
                nc.vector.tensor_scalar_add(y_sb[b][:, js], py[:],
                                            bf_sb[:])
                nc.sync.dma_start(d_y2.ap()[b][:, js], y_sb[b][:, js])

            def emit_final(b):
                for j in range(2):
                    emit_final_j(b, j)

            final_q = []     # deferred emit_final, interleaved into the
            drained = 0      # next sample's unit stream (PE never stalls
            final_at = {}    # on the divide chain at sample boundaries)

            def drain_one():
                nonlocal drained
                pb, pp, pt, pets = pend.pop(0)
                emit_av(pb, pp, pt, pets)
                drained += 1
                if pt == NT - 1:
                    if pb == BPC - 1 and pp == 1:
                        # kernel tail: interleave per-j so the j0 divide
                        # chain overlaps the j1 AV matmuls and epilogue
                        for j in range(2):
                            emit_epi_j(pb, pp, j, tail=True)
                            emit_final_j(pb, j)
                    else:
                        emit_pair_epilogue(pb, pp)
                        if pp == 1:
                            final_q.append(pb)
                            final_at[pb] = drained + 3
                while final_q and (final_at[final_q[0]] <= drained
                                   or not pend):
                    emit_final(final_q.pop(0))

            for b, p, t in units:
                # QK for both heads of the pair.  Head 2 (p==1, s==0) takes
                # the PE rel-add path: identity matmul accumulates the raw
                # Toeplitz rel window into att before QK.
                att = {}
                for s in range(2):
                    h = 2 * p + s
                    addpath = (h == 2)
                    att[s] = ps.tile([128, 32, 32], F32, tag="ps",
                                     name=f"att{s}")
                    relap3 = rel_t[h][:, ds(31 - 4 * t, 32), ds(31, 32)]
                    if addpath:
                        for j in range(2):
                            mm = nc.tensor.matmul(
                                att[s][:, ds(16 * j, 16), :],
                                id_sb[:],
                                rel_t[h][:, ds(31 - 4 * t + 16 * j, 16),
                                         ds(31, 32)],
                                start=True, stop=False,
                            )
                            if j == 1:
                                noldw(mm)
                    row = 64 * p + 32 * s
                    for j in range(2):
                        mm = nc.tensor.matmul(
                            att[s][:, ds(16 * j, 16), :],
                            K_sb[b][ds(row, 32), ts(t, 128)],
                            Q_sb[b][ds(row, 32), ds(512 * j, 512)],
                            start=not addpath, stop=True,
                            tile_position=(row, 0),
                        )
                        if j == 1:
                            noldw(mm)
                # elementwise: E = 4*exp(att) * exp_rel  (rel read through
                # the shared Toeplitz window: cols 1984-252t+63*y2+x2);
                # add-path tiles already hold rel inside att
                ets = {}
                for s in range(2):
                    h = 2 * p + s
                    addpath = (h == 2)
                    relap = rel_t[h][:, ds(31 - 4 * t, 32), ds(31, 32)]
                    if addpath:
                        # att already holds rel + ln4 from the table
                        e_t = et.tile([128, 32, 32], F16, tag="et", name="et")
                        nc.scalar.activation(e_t[:], att[s][:], AF.Exp)
                    elif uidx in sqset:
                        # (att/2 + 1)^2 ~= exp(att); the x4 lives in the table
                        u = ee.tile([128, 32, 32], F16, tag="ee", name="u")
                        nc.vector.tensor_scalar(u[:], att[s][:], 0.5, 1.0,
                                                ALU.mult, ALU.add)
                        sq = et.tile([128, 32, 32], F16, tag="et", name="sq")
                        nc.vector.tensor_mul(sq[:], u[:], u[:])
                        e_t = et.tile([128, 32, 32], F16, tag="et", name="et")
                        nc.vector.tensor_mul(e_t[:], sq[:], relap)
                    else:
                        ex = ee.tile([128, 32, 32], F16, tag="ee", name="ex")
                        nc.scalar.activation(ex[:], att[s][:], AF.Exp)
                        e_t = et.tile([128, 32, 32], F16, tag="et", name="et")
                        eng = nc.gpsimd if uidx in gpmul else nc.vector
                        eng.tensor_mul(e_t[:], ex[:], relap)
                    ets[s] = e_t
                    uidx += 1
                # AV lags `lka` units behind QK (software pipeline: PE
                # never waits on the elementwise chain)
                pend.append((b, p, t, ets))
                if len(pend) > lka:
                    drain_one()
                # warm-keeper: fill residual PE idle so the HAM clock gate
                # stays at 2.4 GHz (and self-heals if it ever drops);
                # rel-add units carry extra real PE work already
                for _ in range(ndm if p == 0 else max(0, ndm - 1)):
                    dummy_mm()
            while pend:
                drain_one()

    nc.compile()
    _CACHE[key] = nc
    return nc


def kernel(x, Wk, bk, Wq, bq, Wv, bv, Wu, bu, pos_enc):
    global LAST_RESULT
    x = np.ascontiguousarray(np.asarray(x, np.float32))
    Wk = np.asarray(Wk, np.float32)
    Wq = np.asarray(Wq, np.float32)
    Wv = np.asarray(Wv, np.float32)
    Wu = np.asarray(Wu, np.float32)
    bq = np.asarray(bq, np.float32)
    bv = np.asarray(bv, np.float32)
    bu = np.asarray(bu, np.float32)
    pos_enc = np.asarray(pos_enc, np.float32)

    wall = np.stack([Wk.T, (Wq * SCALE).T, Wv.T, Wu.T], axis=1)
    wall = np.ascontiguousarray(wall.astype(np.float16))
    bqv = np.ascontiguousarray((bq * SCALE).reshape(E, 1))
    bfv = np.ascontiguousarray((Wu @ bv + bu).reshape(E, 1))

    # compact Toeplitz table: relb[h, p, j] = exp(pos_enc)[h, S(p) + 3968 - j]
    # with S(p) = 63*(p//32) + p%32; kernel reads col 1984-252t+63*y2+x2
    # heads 0,1,3: exp flavor x4 (multiplied in);  head 2: raw flavor + ln4
    # (added into att by the PE identity matmul); the uniform factor 4
    # cancels in the softmax division and keeps exp() biases immediate
    tabP = 4.0 * np.exp(pos_enc)
    tabP[2] = pos_enc[2] + LN4
    fill = np.ones((H, 1, 1), np.float32)
    fill[2] = 0.0
    pidx = np.arange(128)
    S = (63 * (pidx // 32) + pidx % 32)[:, None]  # (128, 1)
    j = np.arange(4032)[None, :]                  # (1, 4032)
    tidx = S + 3968 - j                           # (128, 4032)
    valid = (tidx >= 0) & (tidx < 3969)
    relb = np.where(valid, tabP[:, tidx.clip(0, 3968)], fill)
    relb = np.ascontiguousarray(relb.astype(np.float16))
    ident = np.eye(128, dtype=np.float16)
    sel2 = np.zeros((2, 2, E), np.float16)
    for p in range(2):
        for s in range(2):
            sel2[s, p, 64 * p + 32 * s:64 * p + 32 * s + 32] = 1.0

    nc = _build()

    common = dict(wall=wall, bqv=bqv, bfv=bfv, relb=relb, sel2=sel2,
                  ident=ident)
    in_maps = []
    xr = x.reshape(B, E, N)
    for c in range(NCORES):
        m = dict(common)
        m["x2"] = np.ascontiguousarray(xr[BPC * c:BPC * (c + 1)].astype(np.float16))
        in_maps.append(m)

    trace = os.environ.get("BASS_TRACE", "") not in ("", "0")
    if trace:
        _ensure_ntff_hook()
    res = bass_utils.run_bass_kernel_spmd(
        nc, in_maps, core_ids=list(range(NCORES)), trace=trace)
    LAST_RESULT = res

    y = np.empty((B, E, N), np.float32)
    for c in range(NCORES):
        y[BPC * c:BPC * (c + 1)] = res.results[c]["y2"]
    return y.reshape(B, E, NY, NX)


# revision 80
# speedup vs baseline: 1.0471x; 1.0055x over previous
"""Attention2d Trainium2 kernel.

Reference computation (per sample b):
  K = Wk @ x;  Q = Wq @ x + bq;  V = Wv @ x + bv     (x: [128, 1024])
  per head h (32 channels):  att[k,q] = scale * K_h[:,k].Q_h[:,q] + rel_h[k,q]
  P = softmax_k(att);  out_h = V_h @ P;  y = Wu @ out + bu

Kernel strategy (8 NeuronCores, data-parallel over batch, 2 samples/core):
  - exp(att + rel) = exp(att) * exp(rel).  exp(rel) is gathered host-side
    (exp commutes with the pos_enc gather) and uploaded fp16; the multiply
    runs on the vector/gpsimd engines in SBUF.  This removes the rel-add
    identity matmuls from the PE entirely (1/3 of its matmul work).
  - exp(att) tiles are computed uniformly scaled by 4: ScalarE tiles use
    ACT Exp with bias=ln(4) -> 4*exp(att); a tunable subset of tiles runs
    on the vector engine as (att+2)^2 = 4*(1+att/2)^2 ~= 4*exp(att).  The
    scale cancels exactly in P = E/D, and the DVE path offloads the
    otherwise-bottleneck ScalarE.
  - AV matmul packs a head pair into one [66, N] PSUM accumulator via
    zero-padded stationaries [V_h0|0|ones|0] and [0|V_h1|0|ones]: channel
    rows 0..63 and both softmax denominators D at rows 64..65, so one copy
    evacuates the pair and the reciprocal reads D straight from PSUM.
  - division by D applied pre-Wu via a selector-matmul partition-broadcast
    of 1/D (softmax denominator), bv/bu folded into one final bias.
"""

import math
import os
import sys
import types

sys.path.insert(0, "/opt/trn_rl_repo")

import numpy as np

import concourse.bass as bass
import concourse.tile as tile
from concourse import bacc, mybir
from concourse import bass_utils
from concourse.bass import ds, ts

F32 = mybir.dt.float32
F16 = mybir.dt.float16
AF = mybir.ActivationFunctionType
ALU = mybir.AluOpType

B, E, H, NY, NX = 16, 128, 4, 32, 32
N = NY * NX          # 1024
HC = E // H          # 32
NCORES = 8
BPC = B // NCORES    # 2 samples per core
NT = N // 128        # 8 k-tiles
SCALE = HC ** -0.5
LN4 = math.log(4.0)

LAST_RESULT = None   # BassKernelResults of the most recent run (for test.py)

_CACHE = {}


def _patch_ldw_opt():
    """Enable walrus LDWEIGHTS elision (redundant identity reloads)."""
    if _CACHE.get("ldw_patched"):
        return
    import concourse.bass_utils as _bu
    orig = _bu.run_command

    def patched(argv, **kw):
        argv = [a.replace("--enable-ldw-opt=false", "--enable-ldw-opt=true")
                if isinstance(a, str) else a for a in argv]
        return orig(argv, **kw)

    _bu.run_command = patched
    _CACHE["ldw_patched"] = True


def _ensure_ntff_hook():
    """Register the axon NTFF profile hook that trn_boot couldn't install
    (the image lacks antenv.axon_hooks). Only needed when tracing."""
    if "antenv.axon_hooks" in sys.modules:
        return
    mod = types.ModuleType("antenv.axon_hooks")
    holder = [None]
    mod.set_axon_ntff_profile_hook = lambda h: holder.__setitem__(0, h)
    mod.get_axon_ntff_profile_hook = lambda: holder[0]
    sys.modules["antenv.axon_hooks"] = mod
    try:
        from trn_agent_boot.trn_boot import _ntff_profile_via_ctypes
        mod.set_axon_ntff_profile_hook(
            _ntff_profile_via_ctypes("/opt/axon/libaxon_pjrt.so")
        )
    except Exception:
        pass


def _rel_indices(ny, nx):
    y = np.arange(ny)
    x = np.arange(nx)
    y1, x1, y2, x2 = np.meshgrid(y, x, y, x, indexing="ij")
    idx = (y1 - y2 + ny - 1) * (2 * nx - 1) + (x1 - x2 + nx - 1)
    return idx.reshape(ny * nx, ny * nx)


def _spread(n, total=64):
    """Bresenham-spread set of n tile indices among `total` units."""
    out = set()
    if n <= 0:
        return out
    for i in range(total):
        if (i * n) // total != ((i + 1) * n) // total:
            out.add(i)
    return out


def _build():
    """Build + bacc-compile the per-core program (cached)."""
    if os.environ.get("KLDW", "0") == "1":
        _patch_ldw_opt()
    nsq = int(os.environ.get("KSQ", "0"))     # tiles on the DVE (x+2)^2 path
    ngp = int(os.environ.get("KMG", "8"))     # Sc-path rel-muls on GpSimd
    lka = int(os.environ.get("KLA", "5"))     # AV lookahead (units)
    nwu = int(os.environ.get("KWU", "28"))    # PE warm-up burst matmuls
    ndm = int(os.environ.get("KDM", "3"))     # warm-keeper matmuls per unit
    bfz = bool(int(os.environ.get("KBFZ", "1")))  # final bias known zero
    kdw = int(os.environ.get("KDW", "384"))   # warm-keeper matmul width
    key = ("nc", nsq, ngp, lka, nwu, ndm, bfz, kdw)
    if key in _CACHE:
        return _CACHE[key]
    _is_h2x = lambda i: ((i % 32) // 16 == 1) and (i % 2 == 0)
    sqset = {i for i in _spread(nsq) if not _is_h2x(i)}
    # GpSimd rel-muls: skip the first 12 tiles (6 units) so the pipe fills
    # without waiting on the slow engine; head-2 tiles are PE rel-add path
    _is_h2 = lambda i: ((i % 32) // 16 == 1) and (i % 2 == 0)
    scpath = [i for i in range(12, 64) if i not in sqset and not _is_h2(i)]
    gpmul = {scpath[i] for i in _spread(min(ngp, len(scpath)), len(scpath))}

    nc = bacc.Bacc("TRN2", target_bir_lowering=False, debug=False,
                   num_devices=NCORES)

    d_x2 = nc.dram_tensor("x2", [BPC, E, N], F16, kind="ExternalInput")
    d_wall = nc.dram_tensor("wall", [E, 4, E], F16, kind="ExternalInput")
    d_bq = nc.dram_tensor("bqv", [E, 1], F32, kind="ExternalInput")
    d_bf = nc.dram_tensor("bfv", [E, 1], F32, kind="ExternalInput")
    # compact Toeplitz rel table: T3[h, p, 1984-252t+63*y2+x2] =
    # exp(rel)[h, 128t+p, 32*y2+x2]  (one shared table for all key-tiles t)
    d_rel = nc.dram_tensor("relb", [H, 128, 4032], F16, kind="ExternalInput")
    d_sel = nc.dram_tensor("sel2", [2, 2, E], F16, kind="ExternalInput")
    d_id = nc.dram_tensor("ident", [128, 128], F16, kind="ExternalInput")
    d_y2 = nc.dram_tensor("y2", [BPC, E, N], F32, kind="ExternalOutput")

    def noldw(mm):
        (mm.ins if hasattr(mm, "ins") else mm).ldweights = False

    with nc.allow_low_precision(reason="fp16 matmul operand tiles"), \
         tile.TileContext(nc) as tc:
        with (
            tc.tile_pool(name="const", bufs=1) as const,
            tc.tile_pool(name="persist", bufs=1) as persist,
            tc.tile_pool(name="relp", bufs=1) as relp,
            tc.tile_pool(name="ee", bufs=6 + 2 * lka) as ee,
            tc.tile_pool(name="et", bufs=8 + 2 * lka) as et,
            tc.tile_pool(name="ps", bufs=2, space="PSUM") as ps,
            tc.tile_pool(name="po", bufs=2, space="PSUM") as po,
            tc.tile_pool(name="pw", bufs=1, space="PSUM") as pw,
            tc.tile_pool(name="pf", bufs=1, space="PSUM") as pf,
        ):
            # ---- constants ----
            wall_sb = const.tile([E, 4, E], F16, tag="wall")
            wk_sb = wall_sb[:, 0]
            wq_sb = wall_sb[:, 1]
            wv_sb = wall_sb[:, 2]
            wu_sb = wall_sb[:, 3]
            sel_sb = const.tile([2, 2, E], F16, tag="sel")
            id_sb = const.tile([128, 128], F16, tag="id")
            bq_sb = const.tile([E, 1], F32, tag="bq")
            bf_sb = const.tile([E, 1], F32, tag="bf")
            scr = const.tile([1, 2], F32, tag="scr")
            ln4_sb = const.tile([128, 1], F32, tag="ln4")
            nc.gpsimd.memset(ln4_sb[:], LN4)

            # PE warm-up burst: back-to-back dummy matmuls during the DMA
            # dead time flip the PE HAM clock gate to 2.4 GHz (~3.4us of
            # sustained activity) before the real pipeline starts; the
            # dense pipeline then sustains it.
            wuin = const.tile([128, 512], F16, tag="wuin")
            nc.gpsimd.memset(wuin[:], 0.0)
            pwu = pw.tile([128, 512], F32, tag="pw", name="pwu")

            def dummy_mm(cols=kdw):
                nc.tensor.matmul(pwu[:, 0:cols], wuin[:, 0:128],
                                 wuin[:, 0:cols], start=True, stop=True)

            for i in range(nwu):
                dummy_mm(512)

            x_sb, K_sb, Q_sb, VT_sb, out_sb, D_sb, bcr_sb, y_sb = (
                {}, {}, {}, {}, {}, {}, {}, {})
            for b in range(BPC):
                x_sb[b] = persist.tile([E, N], F16, tag=f"x{b}", name=f"x{b}")
                K_sb[b] = persist.tile([E, N], F16, tag=f"K{b}", name=f"K{b}")
                Q_sb[b] = persist.tile([E, N], F16, tag=f"Q{b}", name=f"Q{b}")
                # AV stationary per (t, pair, s): [128, 66]
                #   s=0: [V_h0 | 0 | 1 0],  s=1: [0 | V_h1 | 0 1]
                VT_sb[b] = persist.tile([128, NT, 2, 2, 66], F16, tag=f"VT{b}",
                                        name=f"VT{b}")
                out_sb[b] = persist.tile([E, N], F16, tag=f"O{b}", name=f"O{b}")
                D_sb[b] = {p: persist.tile([2, N], F16, tag=f"D{b}{p}",
                                           name=f"D{b}{p}") for p in range(2)}
                bcr_sb[b] = persist.tile([E, N], F32, tag=f"bcr{b}",
                                         name=f"bcr{b}")
                y_sb[b] = persist.tile([E, N], F32, tag=f"y{b}", name=f"y{b}")
            rel_t = {}
            for h in range(H):
                rel_t[h] = relp.tile([128, 64, 63], F16, tag=f"rel{h}",
                                     name=f"rel{h}")

            # ---- DMAs: x first on its own (scalar) queue so it lands as
            # early as possible; consts+rel FIFO on the sync queue ----
            for b in range(BPC):
                nc.scalar.dma_start(x_sb[b][:], d_x2.ap()[b])
            nc.sync.dma_start(wall_sb[:], d_wall.ap()[:])
            nc.sync.dma_start(sel_sb[:], d_sel.ap()[:])
            nc.sync.dma_start(id_sb[:], d_id.ap()[:])
            nc.sync.dma_start(bq_sb[:], d_bq.ap()[:])
            nc.sync.dma_start(bf_sb[:], d_bf.ap()[:])
            # rel on the same queue AFTER x/consts (FIFO keeps x first)
            for h in range(H):
                nc.sync.dma_start(
                    rel_t[h][:],
                    d_rel.ap()[h].rearrange("p (y x) -> p y x", x=63))

            # preload the ACT exp table while DMAs run
            nc.vector.memset(scr[:, 0:1], 0.0)
            nc.scalar.activation(scr[:, 1:2], scr[:, 0:1], AF.Exp)
            for b in range(BPC):
                nc.gpsimd.memset(VT_sb[b][:], 0.0)
                nc.gpsimd.memset(VT_sb[b][:, :, :, 0, 64:65], 1.0)
                nc.gpsimd.memset(VT_sb[b][:, :, :, 1, 65:66], 1.0)

            # ---- projections ----
            for b in range(BPC):
                pK = ps.tile([128, N], F32, tag="ps", name=f"pK{b}")
                for j in range(2):
                    js = ds(512 * j, 512)
                    mm = nc.tensor.matmul(pK[:, js], wk_sb, x_sb[b][:, js],
                                          start=True, stop=True)
                    if j == 1:
                        noldw(mm)
                nc.scalar.copy(K_sb[b][:], pK[:])
                pQ = ps.tile([128, N], F32, tag="ps", name=f"pQ{b}")
                for j in range(2):
                    js = ds(512 * j, 512)
                    mm = nc.tensor.matmul(pQ[:, js], wq_sb, x_sb[b][:, js],
                                          start=True, stop=True)
                    if j == 1:
                        noldw(mm)
                nc.vector.tensor_scalar_add(Q_sb[b][:], pQ[:], bq_sb[:])
                # V^T tiles: pV[:, t, p, s, c] = V[channel 64p+32s+c, key 128t+row]
                pV = ps.tile([128, NT, 2, 2, 32], F32, tag="ps", name=f"pV{b}")
                for t in range(NT):
                    nc.tensor.matmul(pV[:, t], x_sb[b][:, ts(t, 128)], wv_sb,
                                     start=True, stop=True)
                nc.vector.tensor_copy(VT_sb[b][:, :, :, 0, 0:32],
                                      pV[:, :, :, 0, :])
                nc.vector.tensor_copy(VT_sb[b][:, :, :, 1, 32:64],
                                      pV[:, :, :, 1, :])

            # ---- attention (units: (b, pair, t); AV lags one unit) ----
            units = [(b, p, t) for b in range(BPC) for p in range(2)
                     for t in range(NT)]
            po2 = {}
            pend = []        # deque of (b, p, t, {s: e_tile})
            uidx = 0

            def emit_av(b, p, t, ets):
                if t == 0:
                    # two per-j accumulators (1 bank each) so the j0 half
                    # is dependency-complete before the j1 stream finishes
                    po2[(b, p)] = [po.tile([66, 512], F32, tag="po",
                                           name=f"po{b}{p}{j}")
                                   for j in range(2)]
                for s in range(2):
                    for j in range(2):
                        mm = nc.tensor.matmul(
                            po2[(b, p)][j][:],
                            VT_sb[b][:, t, p, s, :],
                            ets[s][:, ds(16 * j, 16), :],
                            start=(t == 0 and s == 0),
                            stop=(t == NT - 1 and s == 1),
                        )
                        if j == 1:
                            noldw(mm)

            def emit_epi_j(b, p, j, tail=False):
                # evacuate one 512-col half of the head pair + denominators;
                # at the kernel tail use the (then-idle) scalar engine so
                # these never queue behind the DVE divide chain
                js = ds(512 * j, 512)
                if tail:
                    nc.scalar.copy(out_sb[b][ds(64 * p, 64), js],
                                   po2[(b, p)][j][0:64, :])
                    nc.scalar.copy(D_sb[b][p][:, js],
                                   po2[(b, p)][j][64:66, :])
                else:
                    nc.vector.tensor_copy(out_sb[b][ds(64 * p, 64), js],
                                          po2[(b, p)][j][0:64, :])
                    nc.vector.tensor_copy(D_sb[b][p][:, js],
                                          po2[(b, p)][j][64:66, :])

            def emit_pair_epilogue(b, p):
                for j in range(2):
                    emit_epi_j(b, p, j)

            def emit_final_j(b, j):
                # divide + output projection, one 512-col phase on a
                # dedicated 1-bank PSUM pool (never blocks the att rotation)
                js = ds(512 * j, 512)
                pbc = pf.tile([128, 512], F32, tag="pf", name=f"pbc{b}{j}")
                for p in range(2):
                    nc.tensor.matmul(pbc[:], sel_sb[:, p],
                                     D_sb[b][p][:, js],
                                     start=(p == 0), stop=(p == 1))
                nc.vector.reciprocal_approx_fast(out=bcr_sb[b][:, js],
                                                 in_=pbc[:])
                nc.vector.tensor_mul(out_sb[b][:, js], out_sb[b][:, js],
                                     bcr_sb[b][:, js])
                py = pf.tile([128, 512], F32, tag="pf", name=f"py{b}{j}")
                nc.tensor.matmul(py[:], wu_sb, out_sb[b][:, js],
                                 start=True, stop=True)
                nc.vector.tensor_scalar_add(y_sb[b][:, js], py[:],
                                            bf_sb[:])
                nc.sync.dma_start(d_y2.ap()[b][:, js], y_sb[b][:, js])

            def emit_final(b):
                for j in range(2):
                    emit_final_j(b, j)

            final_q = []     # deferred emit_final, interleaved into the
            drained = 0      # next sample's unit stream (PE never stalls
            final_at = {}    # on the divide chain at sample boundaries)

            def drain_one():
                nonlocal drained
                pb, pp, pt, pets = pend.pop(0)
                emit_av(pb, pp, pt, pets)
                drained += 1
                if pt == NT - 1:
                    if pb == BPC - 1 and pp == 1:
                        # kernel tail: interleave per-j so the j0 divide
                        # chain overlaps the j1 AV matmuls and epilogue
                        for j in range(2):
                            emit_epi_j(pb, pp, j, tail=True)
                            emit_final_j(pb, j)
                    else:
                        emit_pair_epilogue(pb, pp)
                        if pp == 1:
                            final_q.append(pb)
                            final_at[pb] = drained + 3
                while final_q and (final_at[final_q[0]] <= drained
                                   or not pend):
                    emit_final(final_q.pop(0))

            for b, p, t in units:
                # QK for both heads of the pair.  Head 2 (p==1, s==0) takes
                # the PE rel-add path: identity matmul accumulates the raw
                # Toeplitz rel window into att before QK.
                att = {}
                for s in range(2):
                    h = 2 * p + s
                    addpath = (h == 2)
                    att[s] = ps.tile([128, 32, 32], F32, tag="ps",
                                     name=f"att{s}")
                    relap3 = rel_t[h][:, ds(31 - 4 * t, 32), ds(31, 32)]
                    if addpath:
                        for j in range(2):
                            mm = nc.tensor.matmul(
                                att[s][:, ds(16 * j, 16), :],
                                id_sb[:],
                                rel_t[h][:, ds(31 - 4 * t + 16 * j, 16),
                                         ds(31, 32)],
                                start=True, stop=False,
                            )
                            if j == 1:
                                noldw(mm)
                    row = 64 * p + 32 * s
                    for j in range(2):
                        mm = nc.tensor.matmul(
                            att[s][:, ds(16 * j, 16), :],
                            K_sb[b][ds(row, 32), ts(t, 128)],
                            Q_sb[b][ds(row, 32), ds(512 * j, 512)],
                            start=not addpath, stop=True,
                            tile_position=(row, 0),
                        )
                        if j == 1:
                            noldw(mm)
                # elementwise: E = 4*exp(att) * exp_rel  (rel read through
                # the shared Toeplitz window: cols 1984-252t+63*y2+x2);
                # add-path tiles already hold rel inside att
                ets = {}
                for s in range(2):
                    h = 2 * p + s
                    addpath = (h == 2)
                    relap = rel_t[h][:, ds(31 - 4 * t, 32), ds(31, 32)]
                    if addpath:
                        # att already holds rel + ln4 from the table
                        e_t = et.tile([128, 32, 32], F16, tag="et", name="et")
                        nc.scalar.activation(e_t[:], att[s][:], AF.Exp)
                    elif uidx in sqset:
                        # (att/2 + 1)^2 ~= exp(att); the x4 lives in the table
                        u = ee.tile([128, 32, 32], F16, tag="ee", name="u")
                        nc.vector.tensor_scalar(u[:], att[s][:], 0.5, 1.0,
                                                ALU.mult, ALU.add)
                        sq = et.tile([128, 32, 32], F16, tag="et", name="sq")
                        nc.vector.tensor_mul(sq[:], u[:], u[:])
                        e_t = et.tile([128, 32, 32], F16, tag="et", name="et")
                        nc.vector.tensor_mul(e_t[:], sq[:], relap)
                    else:
                        ex = ee.tile([128, 32, 32], F16, tag="ee", name="ex")
                        nc.scalar.activation(ex[:], att[s][:], AF.Exp)
                        e_t = et.tile([128, 32, 32], F16, tag="et", name="et")
                        eng = nc.gpsimd if uidx in gpmul else nc.vector
                        eng.tensor_mul(e_t[:], ex[:], relap)
                    ets[s] = e_t
                    uidx += 1
                # AV lags `lka` units behind QK (software pipeline: PE
                # never waits on the elementwise chain)
                pend.append((b, p, t, ets))
                if len(pend) > lka:
                    drain_one()
                # warm-keeper: fill residual PE idle so the HAM clock gate
                # stays at 2.4 GHz (and self-heals if it ever drops);
                # rel-add units carry extra real PE work already
                for _ in range(ndm if p == 0 else max(0, ndm - 1)):
                    dummy_mm()
            while pend:
                drain_one()

    nc.compile()
    _CACHE[key] = nc
    return nc


def kernel(x, Wk, bk, Wq, bq, Wv, bv, Wu, bu, pos_enc):
    global LAST_RESULT
    x = np.ascontiguousarray(np.asarray(x, np.float32))
    Wk = np.asarray(Wk, np.float32)
    Wq = np.asarray(Wq, np.float32)
    Wv = np.asarray(Wv, np.float32)
    Wu = np.asarray(Wu, np.float32)
    bq = np.asarray(bq, np.float32)
    bv = np.asarray(bv, np.float32)
    bu = np.asarray(bu, np.float32)
    pos_enc = np.asarray(pos_enc, np.float32)

    wall = np.stack([Wk.T, (Wq * SCALE).T, Wv.T, Wu.T], axis=1)
    wall = np.ascontiguousarray(wall.astype(np.float16))
    bqv = np.ascontiguousarray((bq * SCALE).reshape(E, 1))
    bfv = np.ascontiguousarray((Wu @ bv + bu).reshape(E, 1))

    # compact Toeplitz table: relb[h, p, j] = exp(pos_enc)[h, S(p) + 3968 - j]
    # with S(p) = 63*(p//32) + p%32; kernel reads col 1984-252t+63*y2+x2
    # heads 0,1,3: exp flavor x4 (multiplied in);  head 2: raw flavor + ln4
    # (added into att by the PE identity matmul); the uniform factor 4
    # cancels in the softmax division and keeps exp() biases immediate
    tabP = 4.0 * np.exp(pos_enc)
    tabP[2] = pos_enc[2] + LN4
    fill = np.ones((H, 1, 1), np.float32)
    fill[2] = 0.0
    pidx = np.arange(128)
    S = (63 * (pidx // 32) + pidx % 32)[:, None]  # (128, 1)
    j = np.arange(4032)[None, :]                  # (1, 4032)
    tidx = S + 3968 - j                           # (128, 4032)
    valid = (tidx >= 0) & (tidx < 3969)
    relb = np.where(valid, tabP[:, tidx.clip(0, 3968)], fill)
    relb = np.ascontiguousarray(relb.astype(np.float16))
    ident = np.eye(128, dtype=np.float16)
    sel2 = np.zeros((2, 2, E), np.float16)
    for p in range(2):
        for s in range(2):
            sel2[s, p, 64 * p + 32 * s:64 * p + 32 * s + 32] = 1.0

    nc = _build()

    common = dict(wall=wall, bqv=bqv, bfv=bfv, relb=relb, sel2=sel2,
                  ident=ident)
    in_maps = []
    xr = x.reshape(B, E, N)
    for c in range(NCORES):
        m = dict(common)
        m["x2"] = np.ascontiguousarray(xr[BPC * c:BPC * (c + 1)].astype(np.float16))
        in_maps.append(m)

    trace = os.environ.get("BASS_TRACE", "") not in ("", "0")
    if trace:
        _ensure_ntff_hook()
    res = bass_utils.run_bass_kernel_spmd(
        nc, in_maps, core_ids=list(range(NCORES)), trace=trace)
    LAST_RESULT = res

    y = np.empty((B, E, N), np.float32)
    for c in range(NCORES):
        y[BPC * c:BPC * (c + 1)] = res.results[c]["y2"]
    return y.reshape(B, E, NY, NX)
